# revision 32
# baseline (speedup 1.0000x reference)
"""Distributed Trainium2 kernel for AdaptiveSimpleGCNConv.

Math: out = D^{-1/2} (A_set + I) D^{-1/2} @ x @ W.T + b
  A_set: dense 0/1 adjacency from edge_index (duplicates collapse), N=8192.

Strategy (8 NeuronCores, 1D row partition of nodes):
  - Host: dedup edges, compute degree/d=1/sqrt(deg), fold the column scale
    into x' = d*x. Quantize x' to fp8 (hi) plus a 64x-scaled fp8 residual
    (lo). Permute the COLUMN (source-node) space so the columns with the
    largest quantization-error energy come first; the lo correction is only
    applied to the first NLO=16 of 64 column-chunks, leaving the final
    relative error ~1.85e-2 (< 2e-2 gate) while keeping the tensor-engine
    work low (the PE is power-throttled to ~50% duty for ~25% of the run
    when all 8 cores stream fp8 DoubleRow matmuls).
  - Device k: stream adjacency supertiles (fp8, values 0/1/2 exact); for
    each chunk-pair one fp8 DoubleRow matmul (2 contraction chunks per
    instruction) accumulates y_hi per 512-row window, plus a second DR
    matmul into y_lo for the corrected range. Epilogue per window: cast
    y_hi/y_lo to bf16, two 512-row matmuls (lhsT=W.T / W.T/64) accumulate
    out[outfeat, dest] in PSUM, then vector multiply by d[dest] (bf16 row)
    and gpsimd per-partition bias add; output [128, 1024] bf16, host
    transposes + casts.
  - DMA plan: the per-core HBM stream (~9.8MB at ~360GB/s fair share of
    the chip's HBM across 8 cores) is the roofline. The adjacency rides
    the sync queue alone as 9 large-line supertiles (>=4KB lines; >=6KB
    reaches ~26.5 GB/s per DMA engine vs 21.5 at 2KB) issued up-front in
    consumption order; x pieces + consts ride the scalar queue (4 DMAs,
    emitted first). Doorbells stay far ahead and the ~8-deep shared DMA
    semaphore pool never blocks an issue on an incomplete transfer (sem
    reuse always lands on a long-finished DMA). d-scale and bias fold
    into the host-side gather (exact fp32), so the device epilogue is
    just two PSUM->bf16 casts, 4 projection matmuls and 2 output DMAs.
  - No collectives: x planes are replicated to every core by the host.
"""

import sys

sys.path.insert(0, "/opt/trn_rl_repo")

import numpy as np
import ml_dtypes

N = 8192
D = 128
NCORES = 8
RPC = N // NCORES   # 1024 rows per core
NCHUNK = N // 128   # 64 contraction chunks
NPAIR = NCHUNK // 2  # 32 DoubleRow chunk-pairs
NLO = 16            # chunks receiving the lo correction
NLO_PAIR = NLO // 2
NWIN = RPC // 512   # 2 row windows per core
# adjacency supertiles (single sync queue, issued up-front, consumed in
# order): small first slices -> early matmul start; 12-chunk slices in the
# middle (12KB lines -> best per-engine DMA rate); small last -> short tail
SUPERS = [4, 6, 8, 8, 8, 8, 8, 8, 6]
# packed x layout in xc (hi chunk range, lo chunk range per host piece);
# fetched as two slices: cols [0:16) (chunks 0-8 hi+lo) and the rest
XP_HI = [(0, 8), (8, 16), (16, 32), (32, 48), (48, 64)]
XP_LO = [(0, 8), (8, 16), (16, 16), (16, 16), (16, 16)]
XP_LENS = [(h1 - h0) + (l1 - l0)
           for (h0, h1), (l0, l1) in zip(XP_HI, XP_LO)]
XP_STARTS = [sum(XP_LENS[:i]) for i in range(len(XP_LENS))]
XTOT = sum(XP_LENS)  # 80 chunks
XP0_COLS = 16       # first xc slice: chunks 0-8 hi+lo
XP1A_COLS = 16      # second slice: chunks 8-16 hi+lo (cols 16:32)
N_WARM = 4          # dummy matmuls to ramp the PE while first DMAs land
# bf16 const pack: [wT | wT64]; d-scale and bias applied on host
CB_COLS = D + D
S_LO = 64.0         # scale for the lo fp8 plane
BF16 = ml_dtypes.bfloat16
FP8 = ml_dtypes.float8_e4m3fn

_CACHE = {}


def _build_nc():
    from concourse import bacc, bass, tile, mybir

    adt = mybir.dt.float8e4

    nc = bacc.Bacc("TRN2", target_bir_lowering=False, debug=False,
                   num_devices=NCORES)

    adjt_ext = nc.declare_dram_parameter(
        "adjT", [128, NCHUNK, RPC], adt, isOutput=False)
    xc_ext = nc.declare_dram_parameter(
        "xc", [128, XTOT, D], adt, isOutput=False)
    cb_ext = nc.declare_dram_parameter(
        "cb", [128, CB_COLS], mybir.dt.bfloat16, isOutput=False)
    out_ext = nc.declare_dram_parameter(
        "out", [128, NWIN * 512], mybir.dt.bfloat16, isOutput=True)

    DR = mybir.MatmulPerfMode.DoubleRow

    with tile.TileContext(nc) as tc:
        with (
            tc.tile_pool(name="const", bufs=1) as constp,
            tc.tile_pool(name="adj", bufs=1) as adjp,
            tc.tile_pool(name="yt", bufs=2) as ytp,
            tc.tile_pool(name="ot", bufs=2) as otp,
            tc.tile_pool(name="ps_y", bufs=1, space=bass.MemorySpace.PSUM) as psy,
            tc.tile_pool(name="ps_o", bufs=1, space=bass.MemorySpace.PSUM) as pso,
        ):
            # ---- single sync-queue stream, issued up-front in exact
            # consumption order ----
            # x pieces + consts ride the scalar queue (only 4 DMAs, all
            # emitted first so their sems recycle safely); the sync queue
            # carries ONLY the adjacency stream, starting immediately
            xp0 = constp.tile([128, XP0_COLS, D], adt, name="xp0", tag="xp0")
            nc.scalar.dma_start(out=xp0[:], in_=xc_ext[:, 0:XP0_COLS, :])
            xp1a = constp.tile([128, XP1A_COLS, D], adt,
                               name="xp1a", tag="xp1a")
            nc.scalar.dma_start(
                out=xp1a[:], in_=xc_ext[:, XP0_COLS:XP0_COLS + XP1A_COLS, :])
            xp1b = constp.tile(
                [128, XTOT - XP0_COLS - XP1A_COLS, D], adt,
                name="xp1b", tag="xp1b")
            nc.scalar.dma_start(
                out=xp1b[:], in_=xc_ext[:, XP0_COLS + XP1A_COLS:XTOT, :])
            cb = constp.tile([128, CB_COLS], mybir.dt.bfloat16, name="cb")
            nc.scalar.dma_start(out=cb[:], in_=cb_ext[:])

            ats = []
            c0 = 0
            for s, sz in enumerate(SUPERS):
                at = adjp.tile([128, sz, RPC], adt, tag=f"adj{s}",
                               name=f"adj{s}")
                nc.sync.dma_start(out=at[:], in_=adjt_ext[:, c0:c0 + sz, :])
                ats.append((at, c0, sz))
                c0 += sz

            def xview(col):
                if col < XP0_COLS:
                    return xp0, col
                if col < XP0_COLS + XP1A_COLS:
                    return xp1a, col - XP0_COLS
                return xp1b, col - XP0_COLS - XP1A_COLS

            def xsl_hi(q):
                c0 = 2 * q
                i = next(k for k, (h0, h1) in enumerate(XP_HI)
                         if h0 <= c0 < h1)
                t, o = xview(XP_STARTS[i] + (c0 - XP_HI[i][0]))
                return t[:, o:o + 2, :]

            def xsl_lo(q):
                c0 = 2 * q
                i = next(k for k, (l0, l1) in enumerate(XP_LO)
                         if l0 <= c0 < l1)
                t, o = xview(XP_STARTS[i] + (XP_HI[i][1] - XP_HI[i][0])
                             + (c0 - XP_LO[i][0]))
                return t[:, o:o + 2, :]

            ps_hi = [psy.tile([128, 512], mybir.dt.float32, tag=f"pshi{w}",
                              name=f"ps_hi{w}") for w in range(NWIN)]
            ps_lo = [psy.tile([128, 512], mybir.dt.float32, tag=f"pslo{w}",
                              name=f"ps_lo{w}") for w in range(NWIN)]

            scr = constp.tile([128, 512], adt, name="warm_src")
            nc.gpsimd.memset(scr[:], 0)
            ps_w = psy.tile([128, 512], mybir.dt.float32, tag="pswarm",
                            name="ps_warm")
            for _ in range(N_WARM):
                nc.tensor.matmul(ps_w[:], lhsT=scr[:, :128], rhs=scr[:],
                                 start=True, stop=True)

            def mm(q, j, w, at, start, stop):
                cs = slice(2 * j, 2 * j + 2)
                ws = slice(w * 512, (w + 1) * 512)
                nc.tensor.matmul(
                    ps_hi[w][:],
                    lhsT=xsl_hi(q),
                    rhs=at[:, cs, ws],
                    start=start,
                    stop=stop,
                    perf_mode=DR,
                )
                if q < NLO_PAIR:
                    nc.tensor.matmul(
                        ps_lo[w][:],
                        lhsT=xsl_lo(q),
                        rhs=at[:, cs, ws],
                        start=start,
                        stop=(q == NLO_PAIR - 1),
                        perf_mode=DR,
                    )

            yls = [ytp.tile([128, 512], mybir.dt.bfloat16, tag=f"yl{w}",
                            name=f"yl{w}") for w in range(NWIN)]
            ps_os = [pso.tile([128, 512], mybir.dt.float32,
                              name=f"ps_o{w}") for w in range(NWIN)]
            for s, (at, c0, sz) in enumerate(ats):
                q0 = c0 // 2
                last = s == len(ats) - 1
                for j in range(sz // 2):
                    for w in range(NWIN):
                        mm(q0 + j, j, w, at, start=(q0 + j == 0),
                           stop=(last and j == sz // 2 - 1))
                if s == 3:
                    # lo accumulation stopped at pair 7 (inside s2); cast
                    # ps_lo and fold the lo projection into ps_o here, mid
                    # stream, so the end-of-stream chain has ONE matmul left
                    for w in range(NWIN):
                        nc.scalar.copy(yls[w][:], ps_lo[w][:])
                    for w in range(NWIN):
                        nc.tensor.matmul(
                            ps_os[w][:], lhsT=cb[:, D:2 * D], rhs=yls[w][:],
                            start=True, stop=False)

            # end-of-stream epilogue per window: one yh cast, ONE remaining
            # projection matmul (the lo half ran mid-stream), one output
            # cast, one DMA; w0 on vector/sync, w1 on scalar so the two
            # chains run in parallel
            for w in range(NWIN):
                yh = ytp.tile([128, 512], mybir.dt.bfloat16, tag=f"yh{w}")
                if w == 0:
                    nc.vector.tensor_copy(yh[:], ps_hi[w][:])
                else:
                    nc.scalar.copy(yh[:], ps_hi[w][:])
                nc.tensor.matmul(
                    ps_os[w][:], lhsT=cb[:, 0:D], rhs=yh[:],
                    start=False, stop=True)
                ot = otp.tile([128, 512], mybir.dt.bfloat16, tag=f"ot{w}")
                if w == 0:
                    nc.vector.tensor_copy(ot[:], ps_os[w][:])
                else:
                    nc.scalar.copy(ot[:], ps_os[w][:])
                eng = nc.sync if w == 0 else nc.scalar
                eng.dma_start(out=out_ext[:, w * 512:(w + 1) * 512],
                              in_=ot[:])
    nc.compile()
    return nc


def _host_prep(x, edge_index, W, b):
    r = np.asarray(edge_index[0]).astype(np.int64)
    c = np.asarray(edge_index[1]).astype(np.int64)
    uniq = np.unique(r * N + c)
    r_u = uniq // N
    c_u = uniq % N

    degree = np.bincount(r_u, minlength=N).astype(np.float64) + 1.0
    d = (1.0 / np.sqrt(degree)).astype(np.float32)

    xp = np.asarray(x, dtype=np.float32) * d[:, None]
    xh8 = xp.astype(FP8)
    lo = xp - xh8.astype(np.float32)
    xl8 = (lo * S_LO).astype(FP8)

    # permute the column space so the columns with the largest fp8
    # quantization-error energy land in the corrected chunk range [0, NLO)
    order = np.argsort(-(lo * lo).sum(axis=1), kind="stable")
    P = np.empty(N, dtype=np.int64)
    P[order] = np.arange(N)

    def to_chunks(a, nchunk):
        return a.reshape(nchunk, 128, D).transpose(1, 0, 2)  # [128, chk, feat]

    xh_c = to_chunks(xh8[order], NCHUNK)
    xl_c = to_chunks(xl8[order[:NLO * 128]], NLO)
    # packed pieces: [hi range | lo range] per XP_HI/XP_LO (must match device)
    parts = []
    for (h0, h1), (l0, l1) in zip(XP_HI, XP_LO):
        parts.append(xh_c[:, h0:h1])
        if l1 > l0:
            parts.append(xl_c[:, l0:l1])
    xc = np.ascontiguousarray(np.concatenate(parts, axis=1))

    wt = np.asarray(W, dtype=np.float32).T.astype(BF16)
    wt64 = (np.asarray(W, dtype=np.float32).T / S_LO).astype(BF16)
    cb = np.ascontiguousarray(np.concatenate([wt, wt64], axis=1))

    in_maps = []
    for k in range(NCORES):
        mask = (r_u // RPC) == k
        rr = r_u[mask] - k * RPC  # local row in [0, RPC)
        cs = P[c_u[mask]]         # permuted global col in [0, N)
        adjt = np.zeros((128, NCHUNK, RPC), dtype=FP8)
        # adjt[p, cc, q] corresponds to adj[row = q (local), col = cc*128+p]
        adjt[cs & 127, cs >> 7, rr] = 1.0
        jj = np.arange(RPC)
        ii = P[k * RPC + jj]  # permuted diag index -> column
        adjt[ii & 127, ii >> 7, jj] += np.ones(RPC, dtype=FP8)
        in_maps.append({"adjT": adjt, "xc": xc, "cb": cb})
    return in_maps, d


def _gather(res, d, b):
    outs = []
    for k in range(NCORES):
        o = np.asarray(res.results[k]["out"])  # [128 feat, 1024 dest] bf16
        outs.append(o.T.astype(np.float32))
    y = np.concatenate(outs, axis=0)
    return np.ascontiguousarray(
        y * d[:, None] + np.asarray(b, dtype=np.float32)[None, :])


def kernel(x, edge_index, W, b):
    from concourse.bass_utils import run_bass_kernel_spmd

    in_maps, d = _host_prep(x, edge_index, W, b)
    if "nc" not in _CACHE:
        _CACHE["nc"] = _build_nc()
    nc = _CACHE["nc"]
    res = run_bass_kernel_spmd(nc, in_maps, core_ids=list(range(NCORES)))
    return _gather(res, d, b)


if __name__ == "__main__":
    rng = np.random.default_rng(0)
    x = rng.standard_normal((N, D), dtype=np.float32)
    ei = rng.integers(0, N, size=(2, 262144)).astype(np.int64)
    W = rng.standard_normal((D, D), dtype=np.float32) / np.sqrt(D)
    b = rng.standard_normal(D, dtype=np.float32) * 0.01
    out = kernel(x=x, edge_index=ei, W=W, b=b)
    print(out.shape, out.dtype, float(np.abs(out).mean()))


# revision 33
# speedup vs baseline: 1.0824x; 1.0824x over previous
"""Distributed Trainium2 kernel for AdaptiveSimpleGCNConv.

Math: out = D^{-1/2} (A_set + I) D^{-1/2} @ x @ W.T + b
  A_set: dense 0/1 adjacency from edge_index (duplicates collapse), N=8192.

Strategy (8 NeuronCores, 1D row partition of nodes):
  - Host: dedup edges, compute degree/d=1/sqrt(deg), fold the column scale
    into x' = d*x. Quantize x' to fp8 (hi) plus a 64x-scaled fp8 residual
    (lo). Permute the COLUMN (source-node) space so the columns with the
    largest quantization-error energy come first; the lo correction is only
    applied to the first NLO=16 of 64 column-chunks, leaving the final
    relative error ~1.85e-2 (< 2e-2 gate) while keeping the tensor-engine
    work low (the PE is power-throttled to ~50% duty for ~25% of the run
    when all 8 cores stream fp8 DoubleRow matmuls).
  - Device k: stream adjacency supertiles (fp8, values 0/1/2 exact); for
    each chunk-pair one fp8 DoubleRow matmul (2 contraction chunks per
    instruction) accumulates y_hi per 512-row window, plus a second DR
    matmul into y_lo for the corrected range. Epilogue per window: cast
    y_hi/y_lo to bf16, two 512-row matmuls (lhsT=W.T / W.T/64) accumulate
    out[outfeat, dest] in PSUM, then vector multiply by d[dest] (bf16 row)
    and gpsimd per-partition bias add; output [128, 1024] bf16, host
    transposes + casts.
  - DMA plan: the per-core HBM stream (~9.8MB at ~360GB/s fair share of
    the chip's HBM across 8 cores) is the roofline. The adjacency rides
    the sync queue alone as 9 large-line supertiles (>=4KB lines; >=6KB
    reaches ~26.5 GB/s per DMA engine vs 21.5 at 2KB) issued up-front in
    consumption order; x pieces + consts ride the scalar queue (4 DMAs,
    emitted first). Doorbells stay far ahead and the ~8-deep shared DMA
    semaphore pool never blocks an issue on an incomplete transfer (sem
    reuse always lands on a long-finished DMA). d-scale and bias fold
    into the host-side gather (exact fp32), so the device epilogue is
    just two PSUM->bf16 casts, 4 projection matmuls and 2 output DMAs.
  - No collectives: x planes are replicated to every core by the host.
"""

import sys

sys.path.insert(0, "/opt/trn_rl_repo")

import numpy as np
import ml_dtypes

N = 8192
D = 128
NCORES = 8
RPC = N // NCORES   # 1024 rows per core
NCHUNK = N // 128   # 64 contraction chunks
NPAIR = NCHUNK // 2  # 32 DoubleRow chunk-pairs
NLO = 16            # chunks receiving the lo correction
NLO_PAIR = NLO // 2
NWIN = RPC // 512   # 2 row windows per core
# adjacency supertiles (single sync queue, issued up-front, consumed in
# order): small first slices -> early matmul start; 12-chunk slices in the
# middle (12KB lines -> best per-engine DMA rate); small last -> short tail
SUPERS = [4, 6, 8, 8, 8, 8, 8, 8, 6]
# packed x layout in xc (hi chunk range, lo chunk range per host piece);
# fetched as two slices: cols [0:16) (chunks 0-8 hi+lo) and the rest
XP_HI = [(0, 8), (8, 16), (16, 32), (32, 48), (48, 64)]
XP_LO = [(0, 8), (8, 16), (16, 16), (16, 16), (16, 16)]
XP_LENS = [(h1 - h0) + (l1 - l0)
           for (h0, h1), (l0, l1) in zip(XP_HI, XP_LO)]
XP_STARTS = [sum(XP_LENS[:i]) for i in range(len(XP_LENS))]
XTOT = sum(XP_LENS)  # 80 chunks
XP0_COLS = 16       # first xc slice: chunks 0-8 hi+lo
XP1A_COLS = 16      # second slice: chunks 8-16 hi+lo (cols 16:32)
N_WARM = 4          # dummy matmuls to ramp the PE while first DMAs land
# bf16 const pack: [wT | wT64]; d-scale and bias applied on host
CB_COLS = D + D
S_LO = 64.0         # scale for the lo fp8 plane
BF16 = ml_dtypes.bfloat16
FP8 = ml_dtypes.float8_e4m3fn

_CACHE = {}


def _build_nc():
    from concourse import bacc, bass, tile, mybir

    adt = mybir.dt.float8e4

    nc = bacc.Bacc("TRN2", target_bir_lowering=False, debug=False,
                   num_devices=NCORES)

    adjt_ext = nc.declare_dram_parameter(
        "adjT", [128, NCHUNK, RPC], adt, isOutput=False)
    xc_ext = nc.declare_dram_parameter(
        "xc", [128, XTOT, D], adt, isOutput=False)
    cb_ext = nc.declare_dram_parameter(
        "cb", [128, CB_COLS], mybir.dt.bfloat16, isOutput=False)
    out_ext = nc.declare_dram_parameter(
        "out", [128, NWIN * 512], mybir.dt.bfloat16, isOutput=True)

    DR = mybir.MatmulPerfMode.DoubleRow

    with tile.TileContext(nc) as tc:
        with (
            tc.tile_pool(name="const", bufs=1) as constp,
            tc.tile_pool(name="adj", bufs=1) as adjp,
            tc.tile_pool(name="yt", bufs=2) as ytp,
            tc.tile_pool(name="ot", bufs=2) as otp,
            tc.tile_pool(name="ps_y", bufs=1, space=bass.MemorySpace.PSUM) as psy,
            tc.tile_pool(name="ps_o", bufs=2, space=bass.MemorySpace.PSUM) as pso,
        ):
            # ---- single sync-queue stream, issued up-front in exact
            # consumption order ----
            # x pieces + consts ride the scalar queue (only 4 DMAs, all
            # emitted first so their sems recycle safely); the sync queue
            # carries ONLY the adjacency stream, starting immediately
            xp0 = constp.tile([128, XP0_COLS, D], adt, name="xp0", tag="xp0")
            nc.scalar.dma_start(out=xp0[:], in_=xc_ext[:, 0:XP0_COLS, :])
            xp1a = constp.tile([128, XP1A_COLS, D], adt,
                               name="xp1a", tag="xp1a")
            nc.scalar.dma_start(
                out=xp1a[:], in_=xc_ext[:, XP0_COLS:XP0_COLS + XP1A_COLS, :])
            xp1b = constp.tile(
                [128, XTOT - XP0_COLS - XP1A_COLS, D], adt,
                name="xp1b", tag="xp1b")
            nc.scalar.dma_start(
                out=xp1b[:], in_=xc_ext[:, XP0_COLS + XP1A_COLS:XTOT, :])
            cb = constp.tile([128, CB_COLS], mybir.dt.bfloat16, name="cb")
            nc.scalar.dma_start(out=cb[:], in_=cb_ext[:])

            ats = []
            c0 = 0
            for s, sz in enumerate(SUPERS):
                at = adjp.tile([128, sz, RPC], adt, tag=f"adj{s}",
                               name=f"adj{s}")
                nc.sync.dma_start(out=at[:], in_=adjt_ext[:, c0:c0 + sz, :])
                ats.append((at, c0, sz))
                c0 += sz

            def xview(col):
                if col < XP0_COLS:
                    return xp0, col
                if col < XP0_COLS + XP1A_COLS:
                    return xp1a, col - XP0_COLS
                return xp1b, col - XP0_COLS - XP1A_COLS

            def xsl_hi(q):
                c0 = 2 * q
                i = next(k for k, (h0, h1) in enumerate(XP_HI)
                         if h0 <= c0 < h1)
                t, o = xview(XP_STARTS[i] + (c0 - XP_HI[i][0]))
                return t[:, o:o + 2, :]

            def xsl_lo(q):
                c0 = 2 * q
                i = next(k for k, (l0, l1) in enumerate(XP_LO)
                         if l0 <= c0 < l1)
                t, o = xview(XP_STARTS[i] + (XP_HI[i][1] - XP_HI[i][0])
                             + (c0 - XP_LO[i][0]))
                return t[:, o:o + 2, :]

            ps_hi = [psy.tile([128, 512], mybir.dt.float32, tag=f"pshi{w}",
                              name=f"ps_hi{w}") for w in range(NWIN)]
            ps_lo = [psy.tile([128, 512], mybir.dt.float32, tag=f"pslo{w}",
                              name=f"ps_lo{w}") for w in range(NWIN)]

            scr = constp.tile([128, 512], adt, name="warm_src")
            nc.gpsimd.memset(scr[:], 0)
            ps_w = psy.tile([128, 512], mybir.dt.float32, tag="pswarm",
                            name="ps_warm")
            for _ in range(N_WARM):
                nc.tensor.matmul(ps_w[:], lhsT=scr[:, :128], rhs=scr[:],
                                 start=True, stop=True)

            def mm(q, j, w, at, start, stop):
                cs = slice(2 * j, 2 * j + 2)
                ws = slice(w * 512, (w + 1) * 512)
                nc.tensor.matmul(
                    ps_hi[w][:],
                    lhsT=xsl_hi(q),
                    rhs=at[:, cs, ws],
                    start=start,
                    stop=stop,
                    perf_mode=DR,
                )
                if q < NLO_PAIR:
                    nc.tensor.matmul(
                        ps_lo[w][:],
                        lhsT=xsl_lo(q),
                        rhs=at[:, cs, ws],
                        start=start,
                        stop=(q == NLO_PAIR - 1),
                        perf_mode=DR,
                    )

            for s, (at, c0, sz) in enumerate(ats):
                q0 = c0 // 2
                last = s == len(ats) - 1
                for j in range(sz // 2):
                    for w in range(NWIN):
                        mm(q0 + j, j, w, at, start=(q0 + j == 0),
                           stop=(last and j == sz // 2 - 1))

            # epilogue: per window, two 512-row accumulating matmuls project
            # y through W (out[outfeat, dest] in PSUM), then vector x d[dest]
            # and gpsimd + b[outfeat]. lo copies unblock early (~40% in).
            yl0 = ytp.tile([128, 512], mybir.dt.bfloat16, tag="yl0")
            nc.scalar.copy(yl0[:], ps_lo[0][:])
            yl1 = ytp.tile([128, 512], mybir.dt.bfloat16, tag="yl1")
            nc.scalar.copy(yl1[:], ps_lo[1][:])
            yls = [yl0, yl1]
            for w in range(NWIN):
                yh = ytp.tile([128, 512], mybir.dt.bfloat16, tag=f"yh{w}")
                if w == 0:
                    nc.vector.tensor_copy(yh[:], ps_hi[w][:])
                else:
                    nc.scalar.copy(yh[:], ps_hi[w][:])
                yl = yls[w]
                ps_o = pso.tile([128, 512], mybir.dt.float32)
                nc.tensor.matmul(
                    ps_o[:], lhsT=cb[:, 0:D], rhs=yh[:],
                    start=True, stop=False)
                nc.tensor.matmul(
                    ps_o[:], lhsT=cb[:, D:2 * D], rhs=yl[:],
                    start=False, stop=True)
                ot = otp.tile([128, 512], mybir.dt.bfloat16, tag=f"ot{w}")
                if w == 0:
                    nc.vector.tensor_copy(ot[:], ps_o[:])
                else:
                    nc.scalar.copy(ot[:], ps_o[:])
                eng = nc.sync if w == 0 else nc.scalar
                eng.dma_start(out=out_ext[:, w * 512:(w + 1) * 512],
                              in_=ot[:])
    nc.compile()
    return nc


def _host_prep(x, edge_index, W, b):
    r = np.asarray(edge_index[0]).astype(np.int64)
    c = np.asarray(edge_index[1]).astype(np.int64)
    uniq = np.unique(r * N + c)
    r_u = uniq // N
    c_u = uniq % N

    degree = np.bincount(r_u, minlength=N).astype(np.float64) + 1.0
    d = (1.0 / np.sqrt(degree)).astype(np.float32)

    xp = np.asarray(x, dtype=np.float32) * d[:, None]
    xh8 = xp.astype(FP8)
    lo = xp - xh8.astype(np.float32)
    xl8 = (lo * S_LO).astype(FP8)

    # permute the column space so the columns with the largest fp8
    # quantization-error energy land in the corrected chunk range [0, NLO)
    order = np.argsort(-(lo * lo).sum(axis=1), kind="stable")
    P = np.empty(N, dtype=np.int64)
    P[order] = np.arange(N)

    def to_chunks(a, nchunk):
        return a.reshape(nchunk, 128, D).transpose(1, 0, 2)  # [128, chk, feat]

    xh_c = to_chunks(xh8[order], NCHUNK)
    xl_c = to_chunks(xl8[order[:NLO * 128]], NLO)
    # packed pieces: [hi range | lo range] per XP_HI/XP_LO (must match device)
    parts = []
    for (h0, h1), (l0, l1) in zip(XP_HI, XP_LO):
        parts.append(xh_c[:, h0:h1])
        if l1 > l0:
            parts.append(xl_c[:, l0:l1])
    xc = np.ascontiguousarray(np.concatenate(parts, axis=1))

    wt = np.asarray(W, dtype=np.float32).T.astype(BF16)
    wt64 = (np.asarray(W, dtype=np.float32).T / S_LO).astype(BF16)
    cb = np.ascontiguousarray(np.concatenate([wt, wt64], axis=1))

    in_maps = []
    for k in range(NCORES):
        mask = (r_u // RPC) == k
        rr = r_u[mask] - k * RPC  # local row in [0, RPC)
        cs = P[c_u[mask]]         # permuted global col in [0, N)
        adjt = np.zeros((128, NCHUNK, RPC), dtype=FP8)
        # adjt[p, cc, q] corresponds to adj[row = q (local), col = cc*128+p]
        adjt[cs & 127, cs >> 7, rr] = 1.0
        jj = np.arange(RPC)
        ii = P[k * RPC + jj]  # permuted diag index -> column
        adjt[ii & 127, ii >> 7, jj] += np.ones(RPC, dtype=FP8)
        in_maps.append({"adjT": adjt, "xc": xc, "cb": cb})
    return in_maps, d


def _gather(res, d, b):
    outs = []
    for k in range(NCORES):
        o = np.asarray(res.results[k]["out"])  # [128 feat, 1024 dest] bf16
        outs.append(o.T.astype(np.float32))
    y = np.concatenate(outs, axis=0)
    return np.ascontiguousarray(
        y * d[:, None] + np.asarray(b, dtype=np.float32)[None, :])


def kernel(x, edge_index, W, b):
    from concourse.bass_utils import run_bass_kernel_spmd

    in_maps, d = _host_prep(x, edge_index, W, b)
    if "nc" not in _CACHE:
        _CACHE["nc"] = _build_nc()
    nc = _CACHE["nc"]
    res = run_bass_kernel_spmd(nc, in_maps, core_ids=list(range(NCORES)))
    return _gather(res, d, b)


if __name__ == "__main__":
    rng = np.random.default_rng(0)
    x = rng.standard_normal((N, D), dtype=np.float32)
    ei = rng.integers(0, N, size=(2, 262144)).astype(np.int64)
    W = rng.standard_normal((D, D), dtype=np.float32) / np.sqrt(D)
    b = rng.standard_normal(D, dtype=np.float32) * 0.01
    out = kernel(x=x, edge_index=ei, W=W, b=b)
    print(out.shape, out.dtype, float(np.abs(out).mean()))


# revision 34
# speedup vs baseline: 1.1693x; 1.0803x over previous
"""Distributed Trainium2 kernel for AdaptiveSimpleGCNConv.

Math: out = D^{-1/2} (A_set + I) D^{-1/2} @ x @ W.T + b
  A_set: dense 0/1 adjacency from edge_index (duplicates collapse), N=8192.

Strategy (8 NeuronCores, 1D row partition of nodes):
  - Host: dedup edges, compute degree/d=1/sqrt(deg), fold the column scale
    into x' = d*x. Quantize x' to fp8 (hi) plus a 64x-scaled fp8 residual
    (lo). Permute the COLUMN (source-node) space so the columns with the
    largest quantization-error energy come first; the lo correction is only
    applied to the first NLO=16 of 64 column-chunks, leaving the final
    relative error ~1.85e-2 (< 2e-2 gate) while keeping the tensor-engine
    work low (the PE is power-throttled to ~50% duty for ~25% of the run
    when all 8 cores stream fp8 DoubleRow matmuls).
  - Device k: stream adjacency supertiles (fp8, values 0/1/2 exact); for
    each chunk-pair one fp8 DoubleRow matmul (2 contraction chunks per
    instruction) accumulates y_hi per 512-row window, plus a second DR
    matmul into y_lo for the corrected range. Epilogue per window: cast
    y_hi/y_lo to bf16, two 512-row matmuls (lhsT=W.T / W.T/64) accumulate
    out[outfeat, dest] in PSUM, then vector multiply by d[dest] (bf16 row)
    and gpsimd per-partition bias add; output [128, 1024] bf16, host
    transposes + casts.
  - DMA plan: the per-core HBM stream (~9.8MB at ~360GB/s fair share of
    the chip's HBM across 8 cores) is the roofline. The adjacency rides
    the sync queue alone as 9 large-line supertiles (>=4KB lines; >=6KB
    reaches ~26.5 GB/s per DMA engine vs 21.5 at 2KB) issued up-front in
    consumption order; x pieces + consts ride the scalar queue (4 DMAs,
    emitted first). Doorbells stay far ahead and the ~8-deep shared DMA
    semaphore pool never blocks an issue on an incomplete transfer (sem
    reuse always lands on a long-finished DMA). d-scale and bias fold
    into the host-side gather (exact fp32), so the device epilogue is
    just two PSUM->bf16 casts, 4 projection matmuls and 2 output DMAs.
  - No collectives: x planes are replicated to every core by the host.
"""

import sys

sys.path.insert(0, "/opt/trn_rl_repo")

import numpy as np
import ml_dtypes

N = 8192
D = 128
NCORES = 8
RPC = N // NCORES   # 1024 rows per core
NCHUNK = N // 128   # 64 contraction chunks
NPAIR = NCHUNK // 2  # 32 DoubleRow chunk-pairs
NLO = 16            # chunks receiving the lo correction
NLO_PAIR = NLO // 2
NWIN = RPC // 512   # 2 row windows per core
# adjacency supertiles (single sync queue, issued up-front, consumed in
# order): small first slices -> early matmul start; 12-chunk slices in the
# middle (12KB lines -> best per-engine DMA rate); small last -> short tail
SUPERS = [4, 6, 8, 8, 8, 8, 8, 8, 6]
# packed x layout in xc (hi chunk range, lo chunk range per host piece);
# fetched as two slices: cols [0:16) (chunks 0-8 hi+lo) and the rest
XP_HI = [(0, 8), (8, 16), (16, 32), (32, 48), (48, 64)]
XP_LO = [(0, 8), (8, 16), (16, 16), (16, 16), (16, 16)]
XP_LENS = [(h1 - h0) + (l1 - l0)
           for (h0, h1), (l0, l1) in zip(XP_HI, XP_LO)]
XP_STARTS = [sum(XP_LENS[:i]) for i in range(len(XP_LENS))]
XTOT = sum(XP_LENS)  # 80 chunks
XP0_COLS = 16       # first xc slice: chunks 0-8 hi+lo
XP1A_COLS = 16      # second slice: chunks 8-16 hi+lo (cols 16:32)
N_WARM = 4          # dummy matmuls to ramp the PE while first DMAs land
# bf16 const pack: [wT | wT64]; d-scale and bias applied on host
CB_COLS = D + D
S_LO = 64.0         # scale for the lo fp8 plane
BF16 = ml_dtypes.bfloat16
FP8 = ml_dtypes.float8_e4m3fn

_CACHE = {}


def _build_nc():
    from concourse import bacc, bass, tile, mybir

    adt = mybir.dt.float8e4

    nc = bacc.Bacc("TRN2", target_bir_lowering=False, debug=False,
                   num_devices=NCORES)

    adjt_ext = nc.declare_dram_parameter(
        "adjT", [128, NCHUNK, RPC], adt, isOutput=False)
    xc_ext = nc.declare_dram_parameter(
        "xc", [128, XTOT, D], adt, isOutput=False)
    cb_ext = nc.declare_dram_parameter(
        "cb", [128, CB_COLS], mybir.dt.bfloat16, isOutput=False)
    out_ext = nc.declare_dram_parameter(
        "out", [128, NWIN * 512], mybir.dt.bfloat16, isOutput=True)

    DR = mybir.MatmulPerfMode.DoubleRow

    with tile.TileContext(nc) as tc:
        with (
            tc.tile_pool(name="const", bufs=1) as constp,
            tc.tile_pool(name="adj", bufs=1) as adjp,
            tc.tile_pool(name="yt", bufs=2) as ytp,
            tc.tile_pool(name="ot", bufs=2) as otp,
            tc.tile_pool(name="ps_y", bufs=1, space=bass.MemorySpace.PSUM) as psy,
            tc.tile_pool(name="ps_o", bufs=1, space=bass.MemorySpace.PSUM) as pso,
        ):
            # ---- single sync-queue stream, issued up-front in exact
            # consumption order ----
            # x pieces + consts ride the scalar queue (only 4 DMAs, all
            # emitted first so their sems recycle safely); the sync queue
            # carries ONLY the adjacency stream, starting immediately
            xp0 = constp.tile([128, XP0_COLS, D], adt, name="xp0", tag="xp0")
            nc.scalar.dma_start(out=xp0[:], in_=xc_ext[:, 0:XP0_COLS, :])
            xp1a = constp.tile([128, XP1A_COLS, D], adt,
                               name="xp1a", tag="xp1a")
            nc.scalar.dma_start(
                out=xp1a[:], in_=xc_ext[:, XP0_COLS:XP0_COLS + XP1A_COLS, :])
            xp1b = constp.tile(
                [128, XTOT - XP0_COLS - XP1A_COLS, D], adt,
                name="xp1b", tag="xp1b")
            nc.scalar.dma_start(
                out=xp1b[:], in_=xc_ext[:, XP0_COLS + XP1A_COLS:XTOT, :])
            cb = constp.tile([128, CB_COLS], mybir.dt.bfloat16, name="cb")
            nc.scalar.dma_start(out=cb[:], in_=cb_ext[:])

            ats = []
            c0 = 0
            for s, sz in enumerate(SUPERS):
                at = adjp.tile([128, sz, RPC], adt, tag=f"adj{s}",
                               name=f"adj{s}")
                nc.sync.dma_start(out=at[:], in_=adjt_ext[:, c0:c0 + sz, :])
                ats.append((at, c0, sz))
                c0 += sz

            def xview(col):
                if col < XP0_COLS:
                    return xp0, col
                if col < XP0_COLS + XP1A_COLS:
                    return xp1a, col - XP0_COLS
                return xp1b, col - XP0_COLS - XP1A_COLS

            def xsl_hi(q):
                c0 = 2 * q
                i = next(k for k, (h0, h1) in enumerate(XP_HI)
                         if h0 <= c0 < h1)
                t, o = xview(XP_STARTS[i] + (c0 - XP_HI[i][0]))
                return t[:, o:o + 2, :]

            def xsl_lo(q):
                c0 = 2 * q
                i = next(k for k, (l0, l1) in enumerate(XP_LO)
                         if l0 <= c0 < l1)
                t, o = xview(XP_STARTS[i] + (XP_HI[i][1] - XP_HI[i][0])
                             + (c0 - XP_LO[i][0]))
                return t[:, o:o + 2, :]

            ps_hi = [psy.tile([128, 512], mybir.dt.float32, tag=f"pshi{w}",
                              name=f"ps_hi{w}") for w in range(NWIN)]
            ps_lo = [psy.tile([128, 512], mybir.dt.float32, tag=f"pslo{w}",
                              name=f"ps_lo{w}") for w in range(NWIN)]

            scr = constp.tile([128, 512], adt, name="warm_src")
            nc.gpsimd.memset(scr[:], 0)
            ps_w = psy.tile([128, 512], mybir.dt.float32, tag="pswarm",
                            name="ps_warm")
            for _ in range(N_WARM):
                nc.tensor.matmul(ps_w[:], lhsT=scr[:, :128], rhs=scr[:],
                                 start=True, stop=True)

            def mm(q, j, w, at, start, stop):
                cs = slice(2 * j, 2 * j + 2)
                ws = slice(w * 512, (w + 1) * 512)
                nc.tensor.matmul(
                    ps_hi[w][:],
                    lhsT=xsl_hi(q),
                    rhs=at[:, cs, ws],
                    start=start,
                    stop=stop,
                    perf_mode=DR,
                )
                if q < NLO_PAIR:
                    nc.tensor.matmul(
                        ps_lo[w][:],
                        lhsT=xsl_lo(q),
                        rhs=at[:, cs, ws],
                        start=start,
                        stop=(q == NLO_PAIR - 1),
                        perf_mode=DR,
                    )

            yls = [ytp.tile([128, 512], mybir.dt.bfloat16, tag=f"yl{w}",
                            name=f"yl{w}") for w in range(NWIN)]
            ps_os = [pso.tile([128, 512], mybir.dt.float32,
                              name=f"ps_o{w}") for w in range(NWIN)]
            for s, (at, c0, sz) in enumerate(ats):
                q0 = c0 // 2
                last = s == len(ats) - 1
                for j in range(sz // 2):
                    for w in range(NWIN):
                        mm(q0 + j, j, w, at, start=(q0 + j == 0),
                           stop=(last and j == sz // 2 - 1))
                if s == 3:
                    # lo accumulation stopped at pair 7 (inside s2); cast
                    # ps_lo and fold the lo projection into ps_o here, mid
                    # stream, so the end-of-stream chain has ONE matmul left
                    for w in range(NWIN):
                        nc.scalar.copy(yls[w][:], ps_lo[w][:])
                    for w in range(NWIN):
                        nc.tensor.matmul(
                            ps_os[w][:], lhsT=cb[:, D:2 * D], rhs=yls[w][:],
                            start=True, stop=False)

            # end-of-stream epilogue per window: one yh cast, ONE remaining
            # projection matmul (the lo half ran mid-stream), one output
            # cast, one DMA; w0 on vector/sync, w1 on scalar so the two
            # chains run in parallel
            for w in range(NWIN):
                yh = ytp.tile([128, 512], mybir.dt.bfloat16, tag=f"yh{w}")
                if w == 0:
                    nc.vector.tensor_copy(yh[:], ps_hi[w][:])
                else:
                    nc.scalar.copy(yh[:], ps_hi[w][:])
                nc.tensor.matmul(
                    ps_os[w][:], lhsT=cb[:, 0:D], rhs=yh[:],
                    start=False, stop=True)
                ot = otp.tile([128, 512], mybir.dt.bfloat16, tag=f"ot{w}")
                if w == 0:
                    nc.vector.tensor_copy(ot[:], ps_os[w][:])
                else:
                    nc.scalar.copy(ot[:], ps_os[w][:])
                eng = nc.sync if w == 0 else nc.scalar
                eng.dma_start(out=out_ext[:, w * 512:(w + 1) * 512],
                              in_=ot[:])
    nc.compile()
    return nc


def _host_prep(x, edge_index, W, b):
    r = np.asarray(edge_index[0]).astype(np.int64)
    c = np.asarray(edge_index[1]).astype(np.int64)
    uniq = np.unique(r * N + c)
    r_u = uniq // N
    c_u = uniq % N

    degree = np.bincount(r_u, minlength=N).astype(np.float64) + 1.0
    d = (1.0 / np.sqrt(degree)).astype(np.float32)

    xp = np.asarray(x, dtype=np.float32) * d[:, None]
    xh8 = xp.astype(FP8)
    lo = xp - xh8.astype(np.float32)
    xl8 = (lo * S_LO).astype(FP8)

    # permute the column space so the columns with the largest fp8
    # quantization-error energy land in the corrected chunk range [0, NLO)
    order = np.argsort(-(lo * lo).sum(axis=1), kind="stable")
    P = np.empty(N, dtype=np.int64)
    P[order] = np.arange(N)

    def to_chunks(a, nchunk):
        return a.reshape(nchunk, 128, D).transpose(1, 0, 2)  # [128, chk, feat]

    xh_c = to_chunks(xh8[order], NCHUNK)
    xl_c = to_chunks(xl8[order[:NLO * 128]], NLO)
    # packed pieces: [hi range | lo range] per XP_HI/XP_LO (must match device)
    parts = []
    for (h0, h1), (l0, l1) in zip(XP_HI, XP_LO):
        parts.append(xh_c[:, h0:h1])
        if l1 > l0:
            parts.append(xl_c[:, l0:l1])
    xc = np.ascontiguousarray(np.concatenate(parts, axis=1))

    wt = np.asarray(W, dtype=np.float32).T.astype(BF16)
    wt64 = (np.asarray(W, dtype=np.float32).T / S_LO).astype(BF16)
    cb = np.ascontiguousarray(np.concatenate([wt, wt64], axis=1))

    in_maps = []
    for k in range(NCORES):
        mask = (r_u // RPC) == k
        rr = r_u[mask] - k * RPC  # local row in [0, RPC)
        cs = P[c_u[mask]]         # permuted global col in [0, N)
        adjt = np.zeros((128, NCHUNK, RPC), dtype=FP8)
        # adjt[p, cc, q] corresponds to adj[row = q (local), col = cc*128+p]
        adjt[cs & 127, cs >> 7, rr] = 1.0
        jj = np.arange(RPC)
        ii = P[k * RPC + jj]  # permuted diag index -> column
        adjt[ii & 127, ii >> 7, jj] += np.ones(RPC, dtype=FP8)
        in_maps.append({"adjT": adjt, "xc": xc, "cb": cb})
    return in_maps, d


def _gather(res, d, b):
    outs = []
    for k in range(NCORES):
        o = np.asarray(res.results[k]["out"])  # [128 feat, 1024 dest] bf16
        outs.append(o.T.astype(np.float32))
    y = np.concatenate(outs, axis=0)
    return np.ascontiguousarray(
        y * d[:, None] + np.asarray(b, dtype=np.float32)[None, :])


def kernel(x, edge_index, W, b):
    from concourse.bass_utils import run_bass_kernel_spmd

    in_maps, d = _host_prep(x, edge_index, W, b)
    if "nc" not in _CACHE:
        _CACHE["nc"] = _build_nc()
    nc = _CACHE["nc"]
    res = run_bass_kernel_spmd(nc, in_maps, core_ids=list(range(NCORES)))
    return _gather(res, d, b)


if __name__ == "__main__":
    rng = np.random.default_rng(0)
    x = rng.standard_normal((N, D), dtype=np.float32)
    ei = rng.integers(0, N, size=(2, 262144)).astype(np.int64)
    W = rng.standard_normal((D, D), dtype=np.float32) / np.sqrt(D)
    b = rng.standard_normal(D, dtype=np.float32) * 0.01
    out = kernel(x=x, edge_index=ei, W=W, b=b)
    print(out.shape, out.dtype, float(np.abs(out).mean()))
